# revision 1
# baseline (speedup 1.0000x reference)
import sys
sys.path.insert(0, "/opt/trn_rl_repo")
import math
import numpy as np

import concourse.bass as bass
from concourse import bacc, mybir
from concourse.tile import TileContext
from concourse.bass_utils import run_bass_kernel_spmd
from concourse.masks import make_identity

F32 = mybir.dt.float32
F32R = mybir.dt.float32r
I8 = mybir.dt.int8
AF = mybir.ActivationFunctionType
ALU = mybir.AluOpType
AX = mybir.AxisListType

N, G, E = 16384, 32, 524288
D, DFF, ZI, K, L = 512, 1024, 64, 4, 4
UMAP_A, UMAP_B = 1.577, 0.8951
BN_EPS = 1e-5
NCORES = 8
NL = N // NCORES      # 2048 local nodes per core
GL = G // NCORES      # 4 local graphs per core
NG = N // G           # 512 nodes per graph
KB = N // 128         # 128 source blocks

_NC_CACHE = None


def build_nc():
    nc = bacc.Bacc("TRN2", target_bir_lowering=False, debug=False,
                   enable_asserts=True, num_devices=NCORES)

    xt = nc.dram_tensor("xt", (10, NL), F32R, kind="ExternalInput")
    acm = nc.dram_tensor("acm", (N, NL), I8, kind="ExternalInput")
    embw = nc.dram_tensor("embw", (10, D), F32R, kind="ExternalInput")
    gw1 = nc.dram_tensor("gw1", (L * D, D), F32R, kind="ExternalInput")
    gw2 = nc.dram_tensor("gw2", (L * D, D), F32R, kind="ExternalInput")
    mw1 = nc.dram_tensor("mw1", (D, DFF), F32R, kind="ExternalInput")
    mw2 = nc.dram_tensor("mw2", (DFF, DFF), F32R, kind="ExternalInput")
    mw3 = nc.dram_tensor("mw3", (DFF, ZI), F32R, kind="ExternalInput")
    hw1 = nc.dram_tensor("hw1", (K * ZI, ZI), F32R, kind="ExternalInput")
    hw2 = nc.dram_tensor("hw2", (K * ZI, ZI), F32R, kind="ExternalInput")
    gb1_d = nc.dram_tensor("gb1_d", (128, 16), F32, kind="ExternalInput")
    bng_d = nc.dram_tensor("bng_d", (128, 16), F32, kind="ExternalInput")
    bnb_d = nc.dram_tensor("bnb_d", (128, 16), F32, kind="ExternalInput")
    mb1_d = nc.dram_tensor("mb1_d", (128, 8), F32, kind="ExternalInput")
    mb2_d = nc.dram_tensor("mb2_d", (128, 8), F32, kind="ExternalInput")
    mb3_d = nc.dram_tensor("mb3_d", (ZI, 1), F32, kind="ExternalInput")
    hb1_d = nc.dram_tensor("hb1_d", (ZI, K), F32, kind="ExternalInput")
    hb2_d = nc.dram_tensor("hb2_d", (ZI, K), F32, kind="ExternalInput")
    qout = nc.dram_tensor("qout", (GL * K * 4 * 128, NG), F32,
                          kind="ExternalOutput")

    with TileContext(nc) as tc:
        with (
            tc.tile_pool(name="const", bufs=1) as cp,
            tc.tile_pool(name="res", bufs=1) as rp,
            tc.tile_pool(name="ps", bufs=1, space="PSUM") as ps,
            tc.tile_pool(name="dram", bufs=1, space="DRAM") as dp,
        ):
            ident = cp.tile([128, 128], F32, tag="ident")
            make_identity(nc, ident[:])
            nla = cp.tile([128, 1], F32, tag="nla")
            nc.gpsimd.memset(nla[:], -math.log(UMAP_A))
            of = cp.tile([64, 1], F32, tag="of")
            nc.gpsimd.memset(of[:], 1.0)
            onf = cp.tile([1, NG], F32, tag="onf")
            nc.gpsimd.memset(onf[:], 1.0)

            gb1w = cp.tile([128, 16], F32, tag="gb1w")
            nc.sync.dma_start(gb1w[:], gb1_d[:, :])
            bngw = cp.tile([128, 16], F32, tag="bngw")
            nc.sync.dma_start(bngw[:], bng_d[:, :])
            bnbw = cp.tile([128, 16], F32, tag="bnbw")
            nc.sync.dma_start(bnbw[:], bnb_d[:, :])
            mb1w = cp.tile([128, 8], F32, tag="mb1w")
            nc.sync.dma_start(mb1w[:], mb1_d[:, :])
            mb2w = cp.tile([128, 8], F32, tag="mb2w")
            nc.sync.dma_start(mb2w[:], mb2_d[:, :])
            mb3w = cp.tile([ZI, 1], F32, tag="mb3w")
            nc.sync.dma_start(mb3w[:], mb3_d[:, :])
            hb1w = cp.tile([ZI, K], F32, tag="hb1w")
            nc.sync.dma_start(hb1w[:], hb1_d[:, :])
            hb2w = cp.tile([ZI, K], F32, tag="hb2w")
            nc.sync.dma_start(hb2w[:], hb2_d[:, :])

            hT = [rp.tile([128, NL], F32R, tag=f"hT{fc}", name=f"hT{fc}")
                  for fc in range(4)]

            h_slice = [dp.tile([NL, D], F32R, tag=f"hs{i}", name=f"hs{i}")
                       for i in range(4)]
            h_table = [dp.tile([N, D], F32R, tag=f"ht{i}", name=f"ht{i}")
                       for i in range(4)]
            bn_loc = [dp.tile([128, 8], F32, tag=f"bl{i}", name=f"bl{i}")
                      for i in range(L)]
            bn_glob = [dp.tile([128, 8], F32, tag=f"bg{i}", name=f"bg{i}")
                       for i in range(L)]

            # ---------------- embedding ----------------
            with tc.tile_pool(name="emb", bufs=1) as ep:
                xt_sb = ep.tile([10, NL], F32R, tag="xt")
                nc.sync.dma_start(xt_sb[:], xt[:, :])
                ew_sb = ep.tile([10, D], F32R, tag="ew")
                nc.sync.dma_start(ew_sb[:], embw[:, :])
                for i in range(16):
                    p = ps.tile([128, 512], F32, tag=f"b{i % 4}")
                    nc.tensor.matmul(p[:], xt_sb[:, 128 * i:128 * i + 128],
                                     ew_sb[:], start=True, stop=True)
                    hn = ep.tile([128, 512], F32R, tag="hn", bufs=2)
                    nc.vector.tensor_copy(hn[:], p[:])
                    nc.sync.dma_start(h_slice[0][128 * i:128 * i + 128, :],
                                      hn[:])
                for fc in range(4):
                    for j in range(4):
                        p = ps.tile([128, 512], F32, tag=f"b{4 + fc}")
                        nc.tensor.matmul(p[:], ew_sb[:, 128 * fc:128 * fc + 128],
                                         xt_sb[:, 512 * j:512 * j + 512],
                                         start=True, stop=True)
                        nc.vector.tensor_copy(hT[fc][:, 512 * j:512 * j + 512],
                                              p[:])
                nc.gpsimd.collective_compute(
                    "AllGather", ALU.bypass,
                    ins=[h_slice[0][:, :].opt()],
                    outs=[h_table[0][:, :].opt()],
                    replica_groups=[list(range(NCORES))],
                )

            # ---------------- GIN layers ----------------
            with tc.tile_pool(name="gin", bufs=1) as gp:
                for l in range(L):
                    w1s = gp.tile([128, 2048], F32R, tag="w1")
                    w2s = gp.tile([128, 2048], F32R, tag="w2")
                    for ic in range(4):
                        r0 = 512 * l + 128 * ic
                        nc.sync.dma_start(w1s[:, 512 * ic:512 * ic + 512],
                                          gw1[r0:r0 + 128, :])
                        nc.sync.dma_start(w2s[:, 512 * ic:512 * ic + 512],
                                          gw2[r0:r0 + 128, :])
                    mt = [gp.tile([128, NL], F32R, tag=f"mt{fc}", name=f"mt{fc}_{l}")
                          for fc in range(4)]
                    u2 = [gp.tile([128, NL], F32R, tag=f"u2_{fc}", name=f"u2_{fc}_{l}")
                          for fc in range(4)]

                    # aggregation: aggT = h_table.T @ A  (+ hT at evict)
                    for half in range(2):
                        pb = [ps.tile([128, 512], F32, tag=f"b{i}", name=f"pb{i}")
                              for i in range(8)]
                        for k in range(KB):
                            hk_t = gp.tile([128, 512], F32R, tag="hk", bufs=3)
                            nc.sync.dma_start(
                                hk_t[:], h_table[l][128 * k:128 * k + 128, :])
                            ai = gp.tile([128, 1024], I8, tag="ai", bufs=3)
                            nc.sync.dma_start(
                                ai[:], acm[128 * k:128 * k + 128,
                                           1024 * half:1024 * half + 1024])
                            ar = gp.tile([128, 1024], F32R, tag="ar", bufs=3)
                            nc.vector.tensor_copy(ar[:], ai[:])
                            for fc in range(4):
                                for dc in range(2):
                                    nc.tensor.matmul(
                                        pb[fc * 2 + dc][:],
                                        hk_t[:, 128 * fc:128 * fc + 128],
                                        ar[:, 512 * dc:512 * dc + 512],
                                        start=(k == 0), stop=(k == KB - 1))
                        for fc in range(4):
                            for dc in range(2):
                                col = 1024 * half + 512 * dc
                                nc.vector.tensor_tensor(
                                    out=mt[fc][:, col:col + 512],
                                    in0=pb[fc * 2 + dc][:],
                                    in1=hT[fc][:, col:col + 512],
                                    op=ALU.add)

                    # GIN MLP: u1 = relu(m@w1+b1); u2 = u1@w2
                    for j in range(4):
                        ncol = 512 * j
                        u1c = [gp.tile([128, 512], F32R, tag=f"u1_{oc}", bufs=2,
                                        name=f"u1c{oc}") for oc in range(4)]
                        for oc in range(4):
                            p = ps.tile([128, 512], F32, tag=f"b{oc}")
                            for ic in range(4):
                                nc.tensor.matmul(
                                    p[:],
                                    w1s[:, 512 * ic + 128 * oc:
                                        512 * ic + 128 * oc + 128],
                                    mt[ic][:, ncol:ncol + 512],
                                    start=(ic == 0), stop=(ic == 3))
                            nc.scalar.activation(
                                u1c[oc][:], p[:], AF.Relu,
                                bias=gb1w[:, 4 * l + oc:4 * l + oc + 1])
                        for oc in range(4):
                            p = ps.tile([128, 512], F32, tag=f"b{4 + oc}")
                            for ic in range(4):
                                nc.tensor.matmul(
                                    p[:],
                                    w2s[:, 512 * ic + 128 * oc:
                                        512 * ic + 128 * oc + 128],
                                    u1c[ic][:],
                                    start=(ic == 0), stop=(ic == 3))
                            nc.vector.tensor_copy(u2[oc][:, ncol:ncol + 512],
                                                  p[:])

                    # BN stats (local sums) -> AllReduce
                    stat = gp.tile([128, 8], F32, tag="stat")
                    for fc in range(4):
                        nc.vector.reduce_sum(stat[:, fc:fc + 1], u2[fc][:],
                                             axis=AX.X)
                        qacc = gp.tile([128, 1], F32, tag="qacc")
                        for j in range(4):
                            sq = gp.tile([128, 512], F32, tag="sq", bufs=2)
                            nc.scalar.activation(
                                sq[:], u2[fc][:, 512 * j:512 * j + 512],
                                AF.Square)
                            qp = gp.tile([128, 1], F32, tag=f"qp{j}")
                            nc.vector.reduce_sum(qp[:], sq[:], axis=AX.X)
                            if j == 0:
                                nc.vector.tensor_copy(qacc[:], qp[:])
                            else:
                                nc.vector.tensor_tensor(
                                    out=qacc[:], in0=qp[:], in1=qacc[:],
                                    op=ALU.add)
                        nc.vector.tensor_copy(stat[:, 4 + fc:5 + fc], qacc[:])
                    nc.sync.dma_start(bn_loc[l][:, :], stat[:])
                    nc.gpsimd.collective_compute(
                        "AllReduce", ALU.add,
                        ins=[bn_loc[l][:, :].opt()],
                        outs=[bn_glob[l][:, :].opt()],
                        replica_groups=[list(range(NCORES))],
                    )
                    ga = gp.tile([128, 8], F32, tag="ga")
                    nc.sync.dma_start(ga[:], bn_glob[l][:, :])

                    # BN apply + relu + residual (in place into hT)
                    for fc in range(4):
                        mu = gp.tile([128, 1], F32, tag="mu")
                        nc.vector.tensor_scalar(out=mu[:], in0=ga[:, fc:fc + 1],
                                                scalar1=1.0 / N, scalar2=None,
                                                op0=ALU.mult)
                        ex2 = gp.tile([128, 1], F32, tag="ex2")
                        nc.vector.tensor_scalar(out=ex2[:],
                                                in0=ga[:, 4 + fc:5 + fc],
                                                scalar1=1.0 / N, scalar2=None,
                                                op0=ALU.mult)
                        mu2 = gp.tile([128, 1], F32, tag="mu2")
                        nc.vector.tensor_tensor(out=mu2[:], in0=mu[:],
                                                in1=mu[:], op=ALU.mult)
                        var = gp.tile([128, 1], F32, tag="var")
                        nc.vector.tensor_tensor(out=var[:], in0=ex2[:],
                                                in1=mu2[:], op=ALU.subtract)
                        vare = gp.tile([128, 1], F32, tag="vare")
                        nc.vector.tensor_scalar(out=vare[:], in0=var[:],
                                                scalar1=BN_EPS, scalar2=None,
                                                op0=ALU.add)
                        std = gp.tile([128, 1], F32, tag="std")
                        nc.scalar.activation(std[:], vare[:], AF.Sqrt)
                        inv = gp.tile([128, 1], F32, tag="inv")
                        nc.vector.reciprocal(inv[:], std[:])
                        sv = gp.tile([128, 1], F32, tag="sv")
                        nc.vector.tensor_tensor(
                            out=sv[:], in0=inv[:],
                            in1=bngw[:, 4 * l + fc:4 * l + fc + 1],
                            op=ALU.mult)
                        mst = gp.tile([128, 1], F32, tag="mst")
                        nc.vector.tensor_tensor(out=mst[:], in0=mu[:],
                                                in1=sv[:], op=ALU.mult)
                        tv = gp.tile([128, 1], F32, tag="tv")
                        nc.vector.tensor_tensor(
                            out=tv[:], in0=bnbw[:, 4 * l + fc:4 * l + fc + 1],
                            in1=mst[:], op=ALU.subtract)
                        for j in range(4):
                            ncol = 512 * j
                            rt = gp.tile([128, 512], F32R, tag="rt", bufs=2)
                            nc.scalar.activation(
                                rt[:], u2[fc][:, ncol:ncol + 512], AF.Relu,
                                bias=tv[:, 0:1], scale=sv[:, 0:1])
                            nc.vector.tensor_tensor(
                                out=hT[fc][:, ncol:ncol + 512], in0=rt[:],
                                in1=hT[fc][:, ncol:ncol + 512], op=ALU.add)

                    # write updated h back to the replicated table
                    if l < L - 1:
                        for nb in range(16):
                            hn2 = gp.tile([128, 512], F32R, tag="hn2", bufs=2)
                            for fc in range(4):
                                pt = ps.tile([128, 128], F32, tag=f"b{fc}")
                                nc.tensor.transpose(
                                    pt[:],
                                    hT[fc][:, 128 * nb:128 * nb + 128]
                                    .bitcast(F32),
                                    ident[:])
                                nc.vector.tensor_copy(
                                    hn2[:, 128 * fc:128 * fc + 128], pt[:])
                            nc.sync.dma_start(
                                h_slice[l + 1][128 * nb:128 * nb + 128, :],
                                hn2[:])
                        nc.gpsimd.collective_compute(
                            "AllGather", ALU.bypass,
                            ins=[h_slice[l + 1][:, :].opt()],
                            outs=[h_table[l + 1][:, :].opt()],
                            replica_groups=[list(range(NCORES))],
                        )

            # ---------------- final MLP + heads + pairwise ----------------
            with tc.tile_pool(name="fin", bufs=1) as fp:
                mwa = [fp.tile([128, DFF], F32R, tag=f"mw1_{ic}", name=f"mwa{ic}")
                       for ic in range(4)]
                for ic in range(4):
                    nc.sync.dma_start(mwa[ic][:],
                                      mw1[128 * ic:128 * ic + 128, :])
                mwb = [fp.tile([128, DFF], F32R, tag=f"mw2_{ic}", name=f"mwb{ic}")
                       for ic in range(8)]
                for ic in range(8):
                    nc.sync.dma_start(mwb[ic][:],
                                      mw2[128 * ic:128 * ic + 128, :])
                mwc = [fp.tile([128, ZI], F32R, tag=f"mw3_{ic}", name=f"mwc{ic}")
                       for ic in range(8)]
                for ic in range(8):
                    nc.sync.dma_start(mwc[ic][:],
                                      mw3[128 * ic:128 * ic + 128, :])
                hw1s = [fp.tile([ZI, ZI], F32R, tag=f"hw1_{k}", name=f"hw1s{k}")
                        for k in range(K)]
                hw2s = [fp.tile([ZI, ZI], F32R, tag=f"hw2_{k}", name=f"hw2s{k}")
                        for k in range(K)]
                for k in range(K):
                    nc.sync.dma_start(hw1s[k][:], hw1[ZI * k:ZI * k + ZI, :])
                    nc.sync.dma_start(hw2s[k][:], hw2[ZI * k:ZI * k + ZI, :])

                for g in range(GL):
                    gcol = 512 * g
                    z1 = [fp.tile([128, 512], F32R, tag=f"z1_{oc}", name=f"z1_{oc}")
                          for oc in range(8)]
                    for oc in range(8):
                        p = ps.tile([128, 512], F32, tag=f"b{oc}")
                        for ic in range(4):
                            nc.tensor.matmul(
                                p[:],
                                mwa[ic][:, 128 * oc:128 * oc + 128],
                                hT[ic][:, gcol:gcol + 512],
                                start=(ic == 0), stop=(ic == 3))
                        nc.scalar.activation(z1[oc][:], p[:], AF.Relu,
                                             bias=mb1w[:, oc:oc + 1])
                    z2 = [fp.tile([128, 512], F32R, tag=f"z2_{oc}", name=f"z2_{oc}")
                          for oc in range(8)]
                    for oc in range(8):
                        p = ps.tile([128, 512], F32, tag=f"b{oc}")
                        for ic in range(8):
                            nc.tensor.matmul(
                                p[:],
                                mwb[ic][:, 128 * oc:128 * oc + 128],
                                z1[ic][:],
                                start=(ic == 0), stop=(ic == 7))
                        nc.scalar.activation(z2[oc][:], p[:], AF.Relu,
                                             bias=mb2w[:, oc:oc + 1])
                    pz = ps.tile([ZI, 512], F32, tag="b0")
                    for ic in range(8):
                        nc.tensor.matmul(pz[:], mwc[ic][:, 0:ZI], z2[ic][:],
                                         start=(ic == 0), stop=(ic == 7))
                    z3 = fp.tile([ZI, 512], F32R, tag="z3")
                    nc.vector.tensor_tensor(
                        out=z3[:], in0=pz[:],
                        in1=mb3w[:, 0:1].to_broadcast([ZI, 512])[:],
                        op=ALU.add)
                    for k in range(K):
                        p1 = ps.tile([ZI, 512], F32, tag="b1")
                        nc.tensor.matmul(p1[:], hw1s[k][:], z3[:],
                                         start=True, stop=True)
                        h1 = fp.tile([ZI, 512], F32R, tag="h1", bufs=2)
                        nc.scalar.activation(h1[:], p1[:], AF.Relu,
                                             bias=hb1w[:, k:k + 1])
                        p2 = ps.tile([ZI, 512], F32, tag="b2")
                        nc.tensor.matmul(p2[:], hw2s[k][:], h1[:],
                                         start=True, stop=True)
                        hkt = fp.tile([ZI, 512], F32, tag="hkt", bufs=2)
                        nc.vector.tensor_tensor(
                            out=hkt[:], in0=p2[:],
                            in1=hb2w[:, k:k + 1].to_broadcast([ZI, 512])[:],
                            op=ALU.add)
                        hm2 = fp.tile([ZI, 512], F32, tag="hm2", bufs=2)
                        nc.vector.tensor_scalar(out=hm2[:], in0=hkt[:],
                                                scalar1=-2.0, scalar2=None,
                                                op0=ALU.mult)
                        sqt = fp.tile([ZI, 512], F32, tag="sqt", bufs=2)
                        nc.vector.tensor_tensor(out=sqt[:], in0=hkt[:],
                                                in1=hkt[:], op=ALU.mult)
                        pr = ps.tile([1, 512], F32, tag="b3")
                        nc.tensor.matmul(pr[:], of[:], sqt[:],
                                         start=True, stop=True)
                        rsb = fp.tile([1, 512], F32, tag="rsb", bufs=2)
                        nc.vector.tensor_copy(rsb[:], pr[:])
                        for mb in range(4):
                            pd = ps.tile([128, 512], F32, tag=f"b{4 + mb}")
                            nc.tensor.matmul(pd[:],
                                             hm2[:, 128 * mb:128 * mb + 128],
                                             hkt[:], start=True, stop=False)
                            nc.tensor.matmul(pd[:], onf[:, 0:128], rsb[:],
                                             start=False, stop=False,
                                             skip_group_check=True)
                            nc.tensor.matmul(pd[:],
                                             rsb[:, 128 * mb:128 * mb + 128],
                                             onf[:], start=False, stop=True,
                                             skip_group_check=True)
                            d2t = fp.tile([128, 512], F32, tag="d2", bufs=2)
                            nc.vector.tensor_scalar(out=d2t[:], in0=pd[:],
                                                    scalar1=1e-12,
                                                    scalar2=None, op0=ALU.max)
                            lnt = fp.tile([128, 512], F32, tag="ln", bufs=2)
                            nc.scalar.activation(lnt[:], d2t[:], AF.Ln)
                            qt = fp.tile([128, 512], F32, tag="qt", bufs=3)
                            nc.scalar.activation(qt[:], lnt[:], AF.Sigmoid,
                                                 bias=nla[:, 0:1],
                                                 scale=-UMAP_B)
                            row = ((g * K + k) * 4 + mb) * 128
                            nc.sync.dma_start(qout[row:row + 128, :], qt[:])
    nc.compile()
    return nc


def _host_prep(inputs):
    x = np.asarray(inputs["x"], np.float32)
    edge_index = np.asarray(inputs["edge_index"], np.int64)
    src, dst = edge_index[0], edge_index[1]

    shared = {
        "embw": np.ascontiguousarray(np.vstack(
            [np.asarray(inputs["emb_w"], np.float32),
             np.asarray(inputs["emb_b"], np.float32)[None, :]])),
        "gw1": np.ascontiguousarray(
            np.asarray(inputs["gin_w1"], np.float32).reshape(L * D, D)),
        "gw2": np.ascontiguousarray(
            np.asarray(inputs["gin_w2"], np.float32).reshape(L * D, D)),
        "mw1": np.ascontiguousarray(np.asarray(inputs["mlp_w1"], np.float32)),
        "mw2": np.ascontiguousarray(np.asarray(inputs["mlp_w2"], np.float32)),
        "mw3": np.ascontiguousarray(np.asarray(inputs["mlp_w3"], np.float32)),
        "hw1": np.ascontiguousarray(
            np.asarray(inputs["head_w1"], np.float32).reshape(K * ZI, ZI)),
        "hw2": np.ascontiguousarray(
            np.asarray(inputs["head_w2"], np.float32).reshape(K * ZI, ZI)),
        "gb1_d": np.ascontiguousarray(
            np.asarray(inputs["gin_b1"], np.float32)
            .reshape(L, 4, 128).transpose(2, 0, 1).reshape(128, 16)),
        "bng_d": np.ascontiguousarray(
            np.asarray(inputs["bn_g"], np.float32)
            .reshape(L, 4, 128).transpose(2, 0, 1).reshape(128, 16)),
        "bnb_d": np.ascontiguousarray(
            np.asarray(inputs["bn_b"], np.float32)
            .reshape(L, 4, 128).transpose(2, 0, 1).reshape(128, 16)),
        "mb1_d": np.ascontiguousarray(
            np.asarray(inputs["mlp_b1"], np.float32).reshape(8, 128).T),
        "mb2_d": np.ascontiguousarray(
            np.asarray(inputs["mlp_b2"], np.float32).reshape(8, 128).T),
        "mb3_d": np.ascontiguousarray(
            np.asarray(inputs["mlp_b3"], np.float32)[:, None]),
        "hb1_d": np.ascontiguousarray(
            np.asarray(inputs["head_b1"], np.float32).T),
        "hb2_d": np.ascontiguousarray(
            np.asarray(inputs["head_b2"], np.float32).T),
    }

    in_maps = []
    ones_row = np.ones((1, NL), np.float32)
    for c in range(NCORES):
        lo = NL * c
        mask = (dst >= lo) & (dst < lo + NL)
        flat = src[mask] * NL + (dst[mask] - lo)
        a = np.bincount(flat, minlength=N * NL).astype(np.int8)
        m = dict(shared)
        m["acm"] = np.ascontiguousarray(a.reshape(N, NL))
        m["xt"] = np.ascontiguousarray(
            np.vstack([x[lo:lo + NL].T, ones_row]))
        in_maps.append(m)
    return in_maps


def kernel(**inputs) -> np.ndarray:
    global _NC_CACHE
    if _NC_CACHE is None:
        _NC_CACHE = build_nc()
    nc = _NC_CACHE
    in_maps = _host_prep(inputs)
    res = run_bass_kernel_spmd(nc, in_maps, core_ids=list(range(NCORES)))
    out = np.concatenate(
        [np.asarray(res.results[c]["qout"]).reshape(GL, K, NG, NG)
         for c in range(NCORES)], axis=0)
    return out

